# revision 10
# baseline (speedup 1.0000x reference)
"""Trainium2 Bass kernel for nn_DistanceLoss (patch neighbor-distance loss).

Reference semantics (k=16, H=W=2048, LOSS_WEIGHT=1):
  split each image into non-overlapping 16x16 patches; for interior pixels
  (local i,j in 1..14) and the 8-neighbor offset list [E,NW,NE,N,E,SW,SE,S]
  (E twice, W missing), accumulate || |sr_c-sr_n| - |hr_c-hr_n| || and take
  the global mean over L*14*14*8 terms.

Core trick: the per-term value t = ||u|-|v|| (u = sr_c-sr_n, v = hr_c-hr_n)
is three chained ABSOLUTE_DIFF ALU ops. The host stages sr/hr INTERLEAVED
(x[2f]=sr, x[2f+1]=hr) so that in the DVE's fp16 2x packed mode one
instruction sees all four operands per cycle (SRC_0=sr_x, SRC_0_HI=hr_x,
SRC_1=sr_{x+o}, SRC_1_HI=hr_{x+o}) and emits t duplicated to both write
lanes. This toolchain's walrus predates the CUSTOM_DVE_ANT opcodes, so the
custom 3-stage uop program is installed by HIJACKING the stock
TENSOR_TENSOR_ARITH_OP (0x41) row of the per-NEFF DVE table (the stock
sequencer handler already enables the two-source perf mode, which makes the
engine pick the 2x_1P uop slot for fp16 contiguous operands). Every
nc.vector.tensor_tensor in this kernel therefore computes the fused
pair-absdiff, one t per cycle per lane -- there is no S/D construction, no
shifted-copy DMA, no separate abs or min pass, and the Scalar engine is
freed up to issue half the input DMAs.

Opposite offsets +o/-o share one t array (sum over shifted windows), so the
pairs {N,S}, {NW,SE}, {NE,SW} cost one pass each and E (listed twice) has
weight 2. The interior-window sums run on PE as ones/twos-weighted
[128,1]^T @ t-row matmuls accumulating into PSUM [1,224]; rhs APs read the
duplicated t tiles with an inner stride of 2 so each t counts once. The
last pair (E) writes two tiles so PE can start its tail matmuls early.

Sharding: 256 image columns per core (16 patch-cols x 128 patch-rows),
free index f = i*256 + c; every neighbor offset is the constant
interleaved shift 2*(di*256+dj), always 4-byte aligned.
"""

import numpy as np

H = W = 2048
K = 16
NCORES = 8
WC = W // NCORES          # 256 columns per core
FREE = K * WC             # 4096 f-positions per partition
WIN = 15 * WC             # 3840: compute window covers i = 0..14
XPAD = 8208               # x tile width (2*FREE + junk tail for o=257 reads)
N_TERMS = (H // K) * (W // K) * (K - 2) * (K - 2) * 8
SPLIT_A = 1536            # A/B pass boundary (multiple of 256: row-aligned)

TT_ARITH_OPCODE = 0x41    # stock TENSOR_TENSOR_ARITH_OP row we repoint
PAIRMIN_NAME = "PAIRMIN_TT_ANT"


def _split_multiwaits(nc):
    """The walrus build here accepts at most one sync wait (and one update)
    per instruction: hoist extra waits onto same-engine NoOps inserted
    before the instruction, and extra updates onto NoOps after it."""
    from concourse import mybir

    k = 0
    for f in nc.m.functions:
        for bb in f.blocks:
            out, changed = [], False
            for i in bb.instructions:
                si = i.sync_info
                waits = list(si.on_wait) if si else []
                ups = list(si.on_update) if si else []
                trimmed = False
                if len(waits) > 1:
                    for w in waits[:-1]:
                        n = mybir.InstNoOp(name=f"{i.name}-sw{k}", ins=[],
                                           outs=[])
                        k += 1
                        n.engine = i.engine
                        n.sync_info = mybir.SyncInfo(on_wait=[w], on_update=[])
                        out.append(n)
                    waits, changed, trimmed = waits[-1:], True, True
                out.append(i)
                if len(ups) > 1:
                    i.sync_info = mybir.SyncInfo(on_wait=waits,
                                                 on_update=ups[:1])
                    for u in ups[1:]:
                        n = mybir.InstNoOp(name=f"{i.name}-su{k}", ins=[],
                                           outs=[])
                        k += 1
                        n.engine = i.engine
                        n.sync_info = mybir.SyncInfo(on_wait=[], on_update=[u])
                        out.append(n)
                    changed = True
                elif trimmed:
                    i.sync_info = mybir.SyncInfo(on_wait=waits, on_update=ups)
            if changed:
                bb.instructions = out
    return k


def _pairmin_ref(in0, in1, s0, s1, imm2):
    """numpy semantics of the hijacked op (sim/IR reference)."""
    a, b = in0[..., 0::2].astype(np.float32), in0[..., 1::2].astype(np.float32)
    c, d = in1[..., 0::2].astype(np.float32), in1[..., 1::2].astype(np.float32)
    t = np.abs(np.abs(a - c) - np.abs(b - d))
    return np.repeat(t, 2, axis=-1)


def _register_pairmin():
    """Install PAIRMIN into dve_ops.OPS with a hand-built 2x_1P uop program
    keyed to the stock TENSOR_TENSOR opcode row."""
    from concourse.dve_spec import Spec, Src0, Src1, Bin, lower
    from concourse.dve_uop import (
        UopConfig, DveOpSpec, InpSel, OutPath, OutSel,
        AluInp, AluOp, DelayInp, Trigger, ENABLE,
    )
    from concourse.dve_ops import DveOp, OPS, CUSTOM_DVE_SPECS, _COMPILE_CACHE

    if any(op.name == PAIRMIN_NAME for op in OPS):
        return

    u = UopConfig()
    u.inp[0], u.inp_enable[0] = InpSel.SRC_0, ENABLE       # sr_x
    u.inp[1], u.inp_enable[1] = InpSel.SRC_1, ENABLE       # sr_{x+o}
    u.inp[2], u.inp_enable[2] = InpSel.SRC_0_HI, ENABLE    # hr_x
    u.inp[3], u.inp_enable[3] = InpSel.SRC_1_HI, ENABLE    # hr_{x+o}
    dp = u.datapath_config
    dp[0].enable_alu(AluOp.ABSOLUTE_DIFF, AluInp.PREV_ALU_OUT,
                     AluInp.PREV_DELAY_0)
    dp[0].pass_through_delay(1, 2)
    dp[1].enable_alu(AluOp.ABSOLUTE_DIFF, AluInp.PREV_DELAY_1,
                     AluInp.PREV_DELAY_2)
    dp[1].enable_delay_from_src(DelayInp.PREV_ALU_OUT, 0)
    dp[2].enable_alu(AluOp.ABSOLUTE_DIFF, AluInp.PREV_ALU_OUT,
                     AluInp.PREV_DELAY_0)
    for k in range(3, 8):
        dp[k].pass_through_alu()
    u.out[OutPath.WR0_LO], u.out_enable[OutPath.WR0_LO] = OutSel.ALU_OUT, ENABLE
    u.out[OutPath.WR0_HI], u.out_enable[OutPath.WR0_HI] = OutSel.ALU_OUT, ENABLE
    u.require_inp0 = 1
    u.require_inp1 = 1
    u.trigger = (Trigger.SRC_TENSOR_DONE, Trigger.NONE, Trigger.NONE)

    op = DveOp(PAIRMIN_NAME,
               Spec(body=Bin(AluOp.ABSOLUTE_DIFF, Src0, Src1),
                    reference=_pairmin_ref),
               subdim=False, uops_sha={})
    OPS.append(op)
    CUSTOM_DVE_SPECS[PAIRMIN_NAME] = op.spec
    reg = lower(op.spec, ver="v3")
    assert len(reg) == 1
    _COMPILE_CACHE[(PAIRMIN_NAME, "v3")] = DveOpSpec(
        name=PAIRMIN_NAME, opcode=TT_ARITH_OPCODE, uops=reg,
        uops_2x=[u], perf_max=1, rd1_en=True)


def _build_bass():
    from concourse import bass, mybir, tile

    _register_pairmin()

    nc = bass.Bass()
    # block-major staging: 4 DRAM-contiguous blocks of [64 partitions, 4096]
    # (A = cols 0:4096 split by partition halves, then B = cols 4096:8192),
    # giving 8KB sequential-HBM descriptors on both HWDGE queues at once.
    x_in = nc.declare_dram_parameter("x_in", [4 * 64, 4096],
                                     mybir.dt.float16, isOutput=False)
    out_sum = nc.declare_dram_parameter("out_sum", [1, 8],
                                        mybir.dt.float32, isOutput=True)
    nc.m.ant_custom_dve_ops = sorted({*nc.m.ant_custom_dve_ops, PAIRMIN_NAME})

    fp16 = mybir.dt.float16
    f32 = mybir.dt.float32
    Alu = mybir.AluOpType

    with tile.TileContext(nc) as tc:
        with tc.tile_pool(name="io", bufs=1) as io_pool, \
             tc.tile_pool(name="tpool", bufs=4) as t_pool, \
             tc.tile_pool(name="psum", bufs=1, space="PSUM") as psum_pool:
            x = io_pool.tile([128, XPAD], fp16, tag="x")
            w1 = io_pool.tile([128, 1], fp16, tag="w1")
            w2 = io_pool.tile([128, 1], fp16, tag="w2")
            acc = psum_pool.tile([1, 256], f32, tag="acc")
            colsb = io_pool.tile([1, 8], f32, tag="colsb")

            nc.vector.memset(w1[:, :], 1.0)
            nc.vector.memset(w2[:, :], 2.0)

            # 4 block loads (block-major DRAM source), issue split across
            # the two HWDGE queues (sync + scalar). A-phase passes need
            # x[:4096] (blocks 0-1); B-phase needs the rest.
            for c in range(4):
                eng = nc.sync if c % 2 == 0 else nc.scalar
                half, col = (c % 2) * 64, (c // 2) * 4096
                eng.dma_start(out=x[half:half + 64, col:col + 4096],
                              in_=x_in[64 * c:64 * (c + 1), :])

            def rows_w():
                return [(1.0 if (i == 0 or i == 14) else 2.0)
                        for i in range(15)]

            # (offset, window lo, PE plan) in issue order; plan entries:
            # ("mid", j_lo, j_hi, row_weights, row_lo, row_hi) weighted row
            # matmuls, ("emid", ...) the x2-weighted E rows,
            # ("strip", j, row_lo, row_hi) single-column edge matmuls.
            PAIRS = [
                (256, 0, [("mid", 1, 15, rows_w(), 0, 15)]),
                (255, 0, [("mid", 2, 15, rows_w(), 0, 15),
                          ("strip", 1, 1, 15),
                          ("strip", 15, 0, 14)]),
                (257, 0, [("mid", 1, 14, rows_w(), 0, 15),
                          ("strip", 14, 1, 15),
                          ("strip", 0, 0, 14)]),
                (1, WC, [("emid", 1, 15, None, 1, 15)]),
            ]

            first_mm = [True]

            def mm(rhs, wts, stop=False):
                width = int(np.prod(rhs.shape[1:]))
                nc.tensor.matmul(acc[:, 0:width], wts[:, :], rhs,
                                 start=first_mm[0], stop=stop)
                first_mm[0] = False

            # fused pair-absdiff pass over f-window [flo, fhi): one hijacked
            # tensor_tensor on the interleaved tile. dst holds (t,t) pairs.
            def pair_pass(t_tile, tbase, flo, fhi, o):
                nc.vector.tensor_tensor(
                    t_tile[:, 2 * (flo - tbase):2 * (fhi - tbase)],
                    x[:, 2 * flo:2 * fhi],
                    x[:, 2 * (flo + o):2 * (fhi + o)], Alu.add)

            # A phase (needs x[:4096] = chunks 0-3)
            tiles = []
            for o, oplo, plan in PAIRS[:3]:
                t = t_pool.tile([128, 2 * WIN], fp16, tag="t")
                tiles.append(t)
                pair_pass(t, 0, oplo, SPLIT_A, o)
            t_a = t_pool.tile([128, 2 * 2048], fp16, tag="ta")
            t_b1 = t_pool.tile([128, 2 * 1536], fp16, tag="tb1")
            t_b2 = t_pool.tile([128, 2 * 256], fp16, tag="tb2")
            # E rows 1..7 except f=2047 (i=7,j=15, never read by the plan):
            # keeps the A pass inside x[:4096].
            pair_pass(t_a, 0, WC, 2047, 1)

            # B phase (needs the full input). E's rows 8..13 and row 14 go
            # to separate tiles so only one matmul trails the last DVE op.
            for (o, oplo, plan), t in zip(PAIRS[:3], tiles):
                pair_pass(t, 0, SPLIT_A, WIN, o)
            pair_pass(t_b1, 2048, 2048, 3584, 1)
            pair_pass(t_b2, 3584, 3584, WIN, 1)

            # PE reductions. Views: i rows x 16 patches x 16 cols x 2 dups.
            def views(tile_, irows):
                v5 = tile_.rearrange("p (i q j d) -> p i q j d",
                                     q=16, j=16, d=2)
                v4 = tile_.rearrange("p (i q jd) -> p i q jd", q=16, jd=32)
                return v5, v4

            for pi, ((o, oplo, plan), t) in enumerate(
                    zip(PAIRS[:3], tiles)):
                v5, v4 = views(t, 15)
                for e in plan:
                    if e[0] == "mid":
                        _, a, b, wts, rlo, rhi = e
                        for i in range(rlo, rhi):
                            w = w1 if wts[i] == 1.0 else w2
                            mm(v5[:, i, :, a:b, 0:1], w)
                    else:  # ("strip", j, row_lo, row_hi)
                        _, j, rlo, rhi = e
                        mm(v4[:, rlo:rhi, :, 2 * j:2 * j + 1], w1)

            # E: rows 1..7 from t_a (ready after the A phase), 8..13 from
            # t_b1, row 14 from t_b2 (the only matmul after the last TT)
            va5, _ = views(t_a, 8)
            vb5, _ = views(t_b1, 6)
            vc5, _ = views(t_b2, 1)
            for i in range(1, 15):
                v = va5[:, i] if i < 8 else (
                    vb5[:, i - 8] if i < 14 else vc5[:, 0])
                mm(v[:, :, 1:15, 0:1], w2, stop=(i == 14))

            # drain PSUM to a scalar
            nc.vector.tensor_reduce(colsb[:, 0:1], acc[:, 0:224],
                                    mybir.AxisListType.X, Alu.add)
            nc.sync.dma_start(out=out_sum[:, :], in_=colsb[:, :])
    _split_multiwaits(nc)
    return nc


_NC_CACHE = None
LAST_RESULTS = None  # BassKernelResults of the most recent run (for test.py)


def kernel(sr_tensor: np.ndarray, hr_tensor: np.ndarray) -> np.ndarray:
    from concourse.bass_utils import run_bass_kernel_spmd

    global _NC_CACHE, LAST_RESULTS
    if _NC_CACHE is None:
        _NC_CACHE = _build_bass()
    nc = _NC_CACHE

    sr = np.asarray(sr_tensor, dtype=np.float32).reshape(H, W)
    hr = np.asarray(hr_tensor, dtype=np.float32).reshape(H, W)

    in_maps = []
    for c in range(NCORES):
        c0 = c * WC
        # [2048, 256] -> [128 patch-rows, 16 rows, 256 cols] -> interleave
        s16 = sr[:, c0:c0 + WC].reshape(128, FREE).astype(np.float16)
        h16 = hr[:, c0:c0 + WC].reshape(128, FREE).astype(np.float16)
        xi = np.empty((128, FREE, 2), dtype=np.float16)
        xi[:, :, 0] = s16
        xi[:, :, 1] = h16
        xi = xi.reshape(128, 2 * FREE)
        # block-major: [A cols 0:4096 x (parts 0:64 | 64:128)] then B cols
        xs = np.concatenate([xi[0:64, 0:4096], xi[64:128, 0:4096],
                             xi[0:64, 4096:8192], xi[64:128, 4096:8192]],
                            axis=0)
        in_maps.append({"x_in": np.ascontiguousarray(xs)})

    res = run_bass_kernel_spmd(nc, in_maps, list(range(NCORES)))
    LAST_RESULTS = res

    total = 0.0
    for r in res.results:
        total += float(np.asarray(r["out_sum"], dtype=np.float64)[0, 0])
    return np.float32(total / N_TERMS)
